# revision 20
# baseline (speedup 1.0000x reference)
"""Trainium2 Bass kernel for nn_Jitter: 2nd-order-Markov time-jitter gather.

y[b, i, j] = x[b, i, mindex[b, j+1]] for j in [0, T-2), where mindex[b, t]
in {t-1, t, t+1} comes from a Markov chain sampled with jax PRNG (seed).

Strategy
  - Host (CPU jax, bit-identical to the reference): sample the chain,
    reduce it to two {0,1} float32 mask rows per batch row:
        wm[b, j] = 1.0 where the source is column j   (shift -1)
        wp[b, j] = 1.0 where the source is column j+2 (shift +1)
    (base case, ~80% of positions, is the identity shift j+1).
  - Device (8 NeuronCores, 2 batch rows each): stream x through SBUF in
    (128 x ~1K) tiles. PE broadcasts each mask row across 128 partitions
    (K=1 matmul with a ones vector) into PSUM; ACT copies the identity
    diagonal; DVE applies two predicated copies (shift -1 / +1); DMA out.
    The kernel is memory-bound: ~34 MB of HBM traffic per core.
"""

import functools

import numpy as np

B, I, T = 16, 256, 8192
TOUT = T - 2
N_CORES = 8
R = B // N_CORES  # batch rows per core
REPLACE_PROB = 0.1
TW = 2048  # time-chunk width
P = 128


# ----------------------------------------------------------------- host side
def _compute_mindex(seed) -> np.ndarray:
    """(B, T) int32 gather indices, bit-identical to the reference (CPU jax)."""
    import jax
    import jax.numpy as jnp

    cpu = jax.devices("cpu")[0]
    with jax.default_device(cpu):
        p = REPLACE_PROB
        s = 1.0 - 2.0 * p
        tmp = np.tile(np.array([p, s, p], dtype=np.float32), (3, 3, 1))
        tmp[2, 1] = np.array([0.0, s / (p + s), p / (p + s)], dtype=np.float32)
        logits3 = jnp.log(jnp.asarray(tmp))

        n_steps = T - 3
        keys = jax.random.split(jax.random.key(seed), n_steps)

        def step(carry, k):
            prev2, prev1 = carry
            lg = logits3[prev1, prev2]
            smp = jax.random.categorical(k, lg).astype(jnp.int32)
            return (prev1, smp), smp

        init = (jnp.ones(B, jnp.int32), jnp.ones(B, jnp.int32))
        _, samp = jax.lax.scan(step, init, keys)
        mid = samp.T
        ones2 = jnp.ones((B, 2), jnp.int32)
        ones1 = jnp.ones((B, 1), jnp.int32)
        mindex = jnp.concatenate([ones2, mid, ones1], axis=1)
        mindex = mindex + (jnp.arange(T, dtype=jnp.int32) - 1)
        return np.asarray(jax.device_get(mindex))


def _masks_from_mindex(mindex: np.ndarray) -> tuple[np.ndarray, np.ndarray]:
    """(B, TOUT) uint8 {0,1} masks for shift -1 and shift +1."""
    d = mindex - np.arange(T, dtype=np.int32)[None, :]  # in {-1, 0, 1}
    dmid = d[:, 1 : T - 1]
    wm = (dmid == -1).astype(np.uint8)
    wp = (dmid == 1).astype(np.uint8)
    return wm, wp


def _pack_in_maps(x, wm, wp):
    """Per-core input dicts: x shard + combined mask rows (R, 2, TOUT)."""
    in_maps = []
    import ml_dtypes

    for k in range(N_CORES):
        sl = slice(k * R, (k + 1) * R)
        wmp = np.stack([wm[sl], wp[sl]], axis=1)  # (R, 2, TOUT)
        in_maps.append(
            {
                "x": np.ascontiguousarray(x[sl]),
                "wmp": np.ascontiguousarray(wmp.astype(ml_dtypes.bfloat16)),
            }
        )
    return in_maps


# --------------------------------------------------------------- device side
def _build_nc(r=R, i_dim=I, t_dim=T, tw=TW):
    import concourse.bacc as bacc
    import concourse.mybir as mybir
    from concourse.tile import TileContext

    tout = t_dim - 2
    n_half = i_dim // P
    nc = bacc.Bacc(
        "TRN2", target_bir_lowering=False, debug=False, num_devices=N_CORES
    )
    x = nc.dram_tensor("x", [r, i_dim, t_dim], mybir.dt.float32, kind="ExternalInput").ap()
    wmp = nc.dram_tensor(
        "wmp", [r, 2, tout], mybir.dt.bfloat16, kind="ExternalInput"
    ).ap()
    y = nc.dram_tensor("y", [r, i_dim, tout], mybir.dt.float32, kind="ExternalOutput").ap()

    # x viewed as (r, n_half, P, t): one DMA grabs both halves into (P, h, t)
    xv = x.rearrange("r (h p) t -> r p h t", p=P)
    yv = y.rearrange("r (h p) t -> r p h t", p=P)

    n_chunk = (tout + tw - 1) // tw
    with TileContext(nc) as tc:
        with (
            tc.tile_pool(name="xs", bufs=4) as xpool,
            tc.tile_pool(name="ys", bufs=4) as ypool,
            tc.tile_pool(name="stage", bufs=2) as spool,
            tc.tile_pool(name="const", bufs=1) as cpool,
            tc.tile_pool(name="psum", bufs=1, space="PSUM") as ppool,
        ):
            ones = cpool.tile([1, P], mybir.dt.bfloat16)
            nc.gpsimd.memset(ones[:], 1.0)
            for rr in range(r):
                # compact {0,1} mask rows (bf16); PE broadcasts them across
                # partitions into PSUM, so no replicated mask HBM traffic
                st = spool.tile([1, 2, tout], mybir.dt.bfloat16, tag="st")
                nc.sync.dma_start(out=st[:], in_=wmp[rr : rr + 1])
                for c in range(n_chunk):
                    j0 = c * tw
                    w = min(tw, tout - j0)
                    pm = ppool.tile([P, tw], mybir.dt.float32, tag="pm")
                    pp = ppool.tile([P, tw], mybir.dt.float32, tag="pp")
                    for s0 in range(0, w, 512):
                        sw = min(512, w - s0)
                        nc.tensor.matmul(
                            pm[:, s0 : s0 + sw], ones[:],
                            st[:, 0, j0 + s0 : j0 + s0 + sw],
                            start=True, stop=True,
                        )
                    for s0 in range(0, w, 512):
                        sw = min(512, w - s0)
                        nc.tensor.matmul(
                            pp[:, s0 : s0 + sw], ones[:],
                            st[:, 1, j0 + s0 : j0 + s0 + sw],
                            start=True, stop=True,
                        )
                    xt = xpool.tile([P, n_half, tw + 2], mybir.dt.float32, tag="xt")
                    first = rr == 0 and c == 0
                    last = rr == r - 1 and c == n_chunk - 1
                    if first:
                        # split the very first load so compute starts sooner
                        for h in range(n_half):
                            nc.sync.dma_start(
                                out=xt[:, h, : w + 2],
                                in_=xv[rr, :, h, j0 : j0 + w + 2],
                            )
                    else:
                        nc.sync.dma_start(
                            out=xt[:, :, : w + 2], in_=xv[rr, :, :, j0 : j0 + w + 2]
                        )
                    yt = ypool.tile([P, n_half, tw], mybir.dt.float32, tag="yt")
                    for h in range(n_half):
                        nc.scalar.copy(out=yt[:, h, :w], in_=xt[:, h, 1 : w + 1])
                    # all pm readers first, then pp: each mask gets a
                    # rebroadcast window while DVE works on the other.
                    # exception: the last chunk orders h0 fully first so its
                    # half-store can begin one DVE op earlier.
                    if last:
                        for h in range(n_half):
                            nc.vector.copy_predicated(
                                yt[:, h, :w],
                                pm[:, :w].bitcast(mybir.dt.int32),
                                xt[:, h, 0:w],
                            )
                            nc.vector.copy_predicated(
                                yt[:, h, :w],
                                pp[:, :w].bitcast(mybir.dt.int32),
                                xt[:, h, 2 : w + 2],
                            )
                            nc.scalar.dma_start(
                                out=yv[rr, :, h, j0 : j0 + w], in_=yt[:, h, :w]
                            )
                    else:
                        for h in range(n_half):
                            nc.vector.copy_predicated(
                                yt[:, h, :w],
                                pm[:, :w].bitcast(mybir.dt.int32),
                                xt[:, h, 0:w],
                            )
                        for h in range(n_half):
                            nc.vector.copy_predicated(
                                yt[:, h, :w],
                                pp[:, :w].bitcast(mybir.dt.int32),
                                xt[:, h, 2 : w + 2],
                            )
                        nc.scalar.dma_start(
                            out=yv[rr, :, :, j0 : j0 + w], in_=yt[:, :, :w]
                        )
    nc.compile()
    return nc


@functools.lru_cache(maxsize=1)
def _get_nc():
    return _build_nc()


# ------------------------------------------------------------------- wrapper
def _run(x: np.ndarray, seed, trace: bool = False):
    from concourse import bass_utils

    x = np.ascontiguousarray(np.asarray(x, dtype=np.float32))
    assert x.shape == (B, I, T), x.shape
    mindex = _compute_mindex(int(np.asarray(seed)))
    wm, wp = _masks_from_mindex(mindex)

    nc = _get_nc()
    in_maps = _pack_in_maps(x, wm, wp)
    res = bass_utils.run_bass_kernel_spmd(
        nc, in_maps, core_ids=list(range(N_CORES)), trace=trace
    )
    out = np.empty((B, I, TOUT), dtype=np.float32)
    for k in range(N_CORES):
        out[k * R : (k + 1) * R] = res.results[k]["y"]
    return out, res


def kernel(x: np.ndarray, seed) -> np.ndarray:
    out, _ = _run(x, seed, trace=False)
    return out


# revision 21
# speedup vs baseline: 1.1961x; 1.1961x over previous
"""Trainium2 Bass kernel for nn_Jitter: 2nd-order-Markov time-jitter gather.

y[b, i, j] = x[b, i, mindex[b, j+1]] for j in [0, T-2), where mindex[b, t]
in {t-1, t, t+1} comes from a Markov chain sampled with jax PRNG (seed).

Strategy
  - Host (CPU jax, bit-identical to the reference): sample the chain,
    reduce it to two {0,1} float32 mask rows per batch row:
        wm[b, j] = 1.0 where the source is column j   (shift -1)
        wp[b, j] = 1.0 where the source is column j+2 (shift +1)
    (base case, ~80% of positions, is the identity shift j+1).
  - Device (8 NeuronCores, 2 batch rows each): stream x through SBUF in
    (128 x ~1K) tiles. PE broadcasts each mask row across 128 partitions
    (K=1 matmul with a ones vector) into PSUM; ACT copies the identity
    diagonal; DVE applies two predicated copies (shift -1 / +1); DMA out.
    The kernel is memory-bound: ~34 MB of HBM traffic per core.
"""

import functools

import numpy as np

B, I, T = 16, 256, 8192
TOUT = T - 2
N_CORES = 8
R = B // N_CORES  # batch rows per core
REPLACE_PROB = 0.1
TW = 2048  # time-chunk width
P = 128


# ----------------------------------------------------------------- host side
def _compute_mindex(seed) -> np.ndarray:
    """(B, T) int32 gather indices, bit-identical to the reference (CPU jax)."""
    import jax
    import jax.numpy as jnp

    cpu = jax.devices("cpu")[0]
    with jax.default_device(cpu):
        p = REPLACE_PROB
        s = 1.0 - 2.0 * p
        tmp = np.tile(np.array([p, s, p], dtype=np.float32), (3, 3, 1))
        tmp[2, 1] = np.array([0.0, s / (p + s), p / (p + s)], dtype=np.float32)
        logits3 = jnp.log(jnp.asarray(tmp))

        n_steps = T - 3
        keys = jax.random.split(jax.random.key(seed), n_steps)

        def step(carry, k):
            prev2, prev1 = carry
            lg = logits3[prev1, prev2]
            smp = jax.random.categorical(k, lg).astype(jnp.int32)
            return (prev1, smp), smp

        init = (jnp.ones(B, jnp.int32), jnp.ones(B, jnp.int32))
        _, samp = jax.lax.scan(step, init, keys)
        mid = samp.T
        ones2 = jnp.ones((B, 2), jnp.int32)
        ones1 = jnp.ones((B, 1), jnp.int32)
        mindex = jnp.concatenate([ones2, mid, ones1], axis=1)
        mindex = mindex + (jnp.arange(T, dtype=jnp.int32) - 1)
        return np.asarray(jax.device_get(mindex))


def _masks_from_mindex(mindex: np.ndarray) -> tuple[np.ndarray, np.ndarray]:
    """(B, TOUT) uint8 {0,1} masks for shift -1 and shift +1."""
    d = mindex - np.arange(T, dtype=np.int32)[None, :]  # in {-1, 0, 1}
    dmid = d[:, 1 : T - 1]
    wm = (dmid == -1).astype(np.uint8)
    wp = (dmid == 1).astype(np.uint8)
    return wm, wp


def _pack_in_maps(x, wm, wp):
    """Per-core input dicts: x shard + combined mask rows (R, 2, TOUT)."""
    in_maps = []
    import ml_dtypes

    for k in range(N_CORES):
        sl = slice(k * R, (k + 1) * R)
        wmp = np.stack([wm[sl], wp[sl]], axis=1)  # (R, 2, TOUT)
        in_maps.append(
            {
                "x": np.ascontiguousarray(x[sl]),
                "wmp": np.ascontiguousarray(wmp.astype(ml_dtypes.bfloat16)),
            }
        )
    return in_maps


# --------------------------------------------------------------- device side
def _build_nc(r=R, i_dim=I, t_dim=T, tw=TW):
    import concourse.bacc as bacc
    import concourse.mybir as mybir
    from concourse.tile import TileContext

    tout = t_dim - 2
    n_half = i_dim // P
    nc = bacc.Bacc(
        "TRN2", target_bir_lowering=False, debug=False, num_devices=N_CORES
    )
    x = nc.dram_tensor("x", [r, i_dim, t_dim], mybir.dt.float32, kind="ExternalInput").ap()
    wmp = nc.dram_tensor(
        "wmp", [r, 2, tout], mybir.dt.bfloat16, kind="ExternalInput"
    ).ap()
    y = nc.dram_tensor("y", [r, i_dim, tout], mybir.dt.float32, kind="ExternalOutput").ap()

    # x viewed as (r, n_half, P, t): one DMA grabs both halves into (P, h, t)
    xv = x.rearrange("r (h p) t -> r p h t", p=P)
    yv = y.rearrange("r (h p) t -> r p h t", p=P)

    n_chunk = (tout + tw - 1) // tw
    with TileContext(nc) as tc:
        with (
            tc.tile_pool(name="xs", bufs=4) as xpool,
            tc.tile_pool(name="ys", bufs=4) as ypool,
            tc.tile_pool(name="stage", bufs=2) as spool,
            tc.tile_pool(name="const", bufs=1) as cpool,
            tc.tile_pool(name="psum", bufs=1, space="PSUM") as ppool,
        ):
            ones = cpool.tile([1, P], mybir.dt.bfloat16)
            nc.gpsimd.memset(ones[:], 1.0)
            for rr in range(r):
                # compact {0,1} mask rows (bf16); PE broadcasts them across
                # partitions into PSUM, so no replicated mask HBM traffic
                st = spool.tile([1, 2, tout], mybir.dt.bfloat16, tag="st")
                nc.sync.dma_start(out=st[:], in_=wmp[rr : rr + 1])
                for c in range(n_chunk):
                    j0 = c * tw
                    w = min(tw, tout - j0)
                    pm = ppool.tile([P, tw], mybir.dt.float32, tag="pm")
                    pp = ppool.tile([P, tw], mybir.dt.float32, tag="pp")
                    for s0 in range(0, w, 512):
                        sw = min(512, w - s0)
                        nc.tensor.matmul(
                            pm[:, s0 : s0 + sw], ones[:],
                            st[:, 0, j0 + s0 : j0 + s0 + sw],
                            start=True, stop=True,
                        )
                    for s0 in range(0, w, 512):
                        sw = min(512, w - s0)
                        nc.tensor.matmul(
                            pp[:, s0 : s0 + sw], ones[:],
                            st[:, 1, j0 + s0 : j0 + s0 + sw],
                            start=True, stop=True,
                        )
                    xt = xpool.tile([P, n_half, tw + 2], mybir.dt.float32, tag="xt")
                    nc.sync.dma_start(
                        out=xt[:, :, : w + 2], in_=xv[rr, :, :, j0 : j0 + w + 2]
                    )
                    yt = ypool.tile([P, n_half, tw], mybir.dt.float32, tag="yt")
                    for h in range(n_half):
                        nc.scalar.copy(out=yt[:, h, :w], in_=xt[:, h, 1 : w + 1])
                    # all pm readers first, then pp: each mask gets a
                    # rebroadcast window while DVE works on the other
                    for h in range(n_half):
                        nc.vector.copy_predicated(
                            yt[:, h, :w],
                            pm[:, :w].bitcast(mybir.dt.int32),
                            xt[:, h, 0:w],
                        )
                    for h in range(n_half):
                        nc.vector.copy_predicated(
                            yt[:, h, :w],
                            pp[:, :w].bitcast(mybir.dt.int32),
                            xt[:, h, 2 : w + 2],
                        )
                    nc.scalar.dma_start(
                        out=yv[rr, :, :, j0 : j0 + w], in_=yt[:, :, :w]
                    )
    nc.compile()
    return nc


@functools.lru_cache(maxsize=1)
def _get_nc():
    return _build_nc()


# ------------------------------------------------------------------- wrapper
def _run(x: np.ndarray, seed, trace: bool = False):
    from concourse import bass_utils

    x = np.ascontiguousarray(np.asarray(x, dtype=np.float32))
    assert x.shape == (B, I, T), x.shape
    mindex = _compute_mindex(int(np.asarray(seed)))
    wm, wp = _masks_from_mindex(mindex)

    nc = _get_nc()
    in_maps = _pack_in_maps(x, wm, wp)
    res = bass_utils.run_bass_kernel_spmd(
        nc, in_maps, core_ids=list(range(N_CORES)), trace=trace
    )
    out = np.empty((B, I, TOUT), dtype=np.float32)
    for k in range(N_CORES):
        out[k * R : (k + 1) * R] = res.results[k]["y"]
    return out, res


def kernel(x: np.ndarray, seed) -> np.ndarray:
    out, _ = _run(x, seed, trace=False)
    return out
